# revision 31
# baseline (speedup 1.0000x reference)
"""VQ codebook squared-distance kernel for Trainium2 (8 NeuronCores).

Computes dist[n,k,l] = (||x[n,:,l]||^2 + ||w[k,:]||^2 - 2*x[n,:,l].w[k,:]) / scale^2
for x (32,128,3136) f32, weight (64,128) f32, scale (1,) f32 -> out (32,64,3136) f32.

Sharding: data-parallel over N (4 per core); weight/scale replicated.
The kernel is HBM-bound: 9.64 MB/core over a stack shared with the paired
core caps at ~310 GB/s, so the structure exists to keep the DMA stream
saturated; all compute hides under it.

Per-core design (fp16 PE path):
  - inputs: 8 fp32 half-tiles, all on the sync HWDGE ring (clean trigger
    FIFO); outputs on the scalar ring (disjoint trigger FIFO).
  - DVE casts x -> fp16 (2x_2P); ACT computes x^2 -> fp16 (Square, fp32 in).
  - PE: psum = (-2Wt)f16 @ x_f16 + ones_f16 @ (x^2)_f16, two n's per PSUM
    tile via column tiling (tile_position (0,0)/(0,64)); psum tiles span
    2 banks so one DVE epilogue covers 1024 cols:
    out = (psum + ||c_k||^2) / scale^2.
"""

import numpy as np

N, D, L, K = 32, 128, 3136, 64
N_CORES = 8
NS = N // N_CORES          # n's per core
LC = 392                   # matmul chunk (8 per row, one PSUM bank)
LH = L // 2                # half length for input DMA

_cache = {}


def _build():
    import concourse.bacc as bacc
    import concourse.mybir as mybir
    import concourse.tile as tile
    from concourse.masks import make_identity

    f32 = mybir.dt.float32
    f16 = mybir.dt.float16
    AF = mybir.ActivationFunctionType

    nc = bacc.Bacc(
        "TRN2",
        target_bir_lowering=False,
        debug=False,
        enable_asserts=False,
        num_devices=N_CORES,
    )

    x_ap = nc.dram_tensor("x", (NS, D, L), f32, kind="ExternalInput").ap()
    w_ap = nc.dram_tensor("weight", (K, D), f32, kind="ExternalInput").ap()
    s_ap = nc.dram_tensor("scale", (1,), f32, kind="ExternalInput").ap()
    o_ap = nc.dram_tensor("out", (NS, K, L), f32, kind="ExternalOutput").ap()

    with tile.TileContext(nc) as tc:
        with (
            tc.tile_pool(name="consts", bufs=1) as consts,
            tc.tile_pool(name="xin", bufs=4) as xpool,
            tc.tile_pool(name="xsq", bufs=4) as xqpool,
            tc.tile_pool(name="outp", bufs=2) as opool,
            tc.tile_pool(name="psum", bufs=4, space="PSUM") as pspool,
            tc.tile_pool(name="psum1", bufs=1, space="PSUM") as pspool1,
        ):
            # ---- input stream: SWDGE cast-on-load fp32->fp16 halves --------
            xts = []
            for n in range(NS):
                xt = xpool.tile([D, L], f16, tag="xt", name=f"x_{n}")
                for h in range(2):
                    hs = slice(h * LH, (h + 1) * LH)
                    nc.gpsimd.dma_start(out=xt[:, hs], in_=x_ap[n][:, hs])
                xts.append(xt)

            # ---- constants -------------------------------------------------
            w2 = consts.tile([2 * K, D], f32)
            nc.sync.dma_start(out=w2[0:K, :], in_=w_ap)
            nc.sync.dma_start(out=w2[K : 2 * K, :], in_=w_ap)

            s_b = consts.tile([128, 1], f32)
            nc.gpsimd.dma_start(out=s_b, in_=s_ap.to_broadcast((128, 1)))
            inv_s2 = consts.tile([128, 1], f32)
            nc.vector.tensor_mul(inv_s2, s_b, s_b)
            nc.vector.reciprocal(inv_s2, inv_s2)

            ident = consts.tile([K, K], f32)
            make_identity(nc, ident)
            ps_w = pspool1.tile([D, K], f32)
            nc.tensor.transpose(ps_w, w2[0:K, :], ident)
            wT16 = consts.tile([D, K], f16)
            nc.vector.tensor_scalar_mul(wT16, in0=ps_w, scalar1=-2.0)

            ones16 = consts.tile([D, K], f16)
            nc.vector.memset(ones16, 1.0)

            w_sq = consts.tile([2 * K, D], f32)
            nc.vector.tensor_mul(w_sq, w2, w2)
            c_sq = consts.tile([2 * K, 1], f32)
            nc.vector.reduce_sum(out=c_sq, in_=w_sq, axis=mybir.AxisListType.X)

            # ---- derived stream: fp16 x^2 per n (from fp16 x) -------------
            xqs = []
            for n in range(NS):
                xq = xqpool.tile([D, L], f16, tag="xq", name=f"xsq_{n}")
                for h in range(2):
                    hs = slice(h * LH, (h + 1) * LH)
                    nc.scalar.activation(xq[:, hs], xts[n][:, hs], AF.Square)
                xqs.append(xq)

            # ---- matmuls + epilogue per pair ------------------------------
            rings = [nc.sync, nc.scalar]
            ring_i = 0
            for pair in range(NS // 2):
                n0, n1 = 2 * pair, 2 * pair + 1
                out_t = opool.tile([2 * K, L], f32, tag="out_t", name=f"out_{pair}")
                for c in range(L // LC):
                    sl = slice(c * LC, (c + 1) * LC)
                    ps = pspool.tile([2 * K, LC], f32, name="ps")
                    nc.tensor.matmul(
                        ps[0:K, :], wT16, xts[n0][:, sl],
                        start=True, stop=False, tile_position=(0, 0),
                    )
                    nc.tensor.matmul(
                        ps[K : 2 * K, :], wT16, xts[n1][:, sl],
                        start=True, stop=False, tile_position=(0, 64),
                    )
                    nc.tensor.matmul(
                        ps[0:K, :], ones16, xqs[n0][:, sl],
                        start=False, stop=True, tile_position=(0, 0),
                    )
                    nc.tensor.matmul(
                        ps[K : 2 * K, :], ones16, xqs[n1][:, sl],
                        start=False, stop=True, tile_position=(0, 64),
                    )
                    nc.vector.tensor_scalar(
                        out=out_t[:, sl], in0=ps,
                        scalar1=c_sq, scalar2=inv_s2,
                        op0=mybir.AluOpType.add, op1=mybir.AluOpType.mult,
                    )
                o_pair = o_ap[2 * pair : 2 * pair + 2].rearrange("a k l -> (a k) l")
                for h in range(2):
                    hs = slice(h * LH, (h + 1) * LH)
                    rings[ring_i % 2].dma_start(out=o_pair[:, hs], in_=out_t[:, hs])
                    ring_i += 1

    nc.compile()
    return nc


def _get_nc():
    if "nc" not in _cache:
        _cache["nc"] = _build()
    return _cache["nc"]


def run(x, weight, scale, trace=False):
    from concourse.bass_utils import run_bass_kernel_spmd

    x = np.ascontiguousarray(np.asarray(x, dtype=np.float32))
    weight = np.ascontiguousarray(np.asarray(weight, dtype=np.float32))
    scale = np.ascontiguousarray(np.asarray(scale, dtype=np.float32))
    assert x.shape == (N, D, L) and weight.shape == (K, D) and scale.shape == (1,)

    nc = _get_nc()
    in_maps = [
        {"x": x[c * NS : (c + 1) * NS], "weight": weight, "scale": scale}
        for c in range(N_CORES)
    ]
    res = run_bass_kernel_spmd(
        nc, in_maps, core_ids=list(range(N_CORES)), trace=trace
    )
    out = np.concatenate([r["out"] for r in res.results], axis=0)
    return out, res


def kernel(x, weight, scale):
    out, _ = run(x, weight, scale, trace=False)
    return out
